# revision 4
# baseline (speedup 1.0000x reference)
# Trainium2 Bass kernel for nn_MultiHeadAttention_24902220382931.
#
# Strategy: data-parallel over sentences. The 32 variable-length sentences are
# sorted by length; core c processes ranks {c, 15-c, 16+c, 31-c} (exactly equal
# token counts, near-equal attention work). Each core packs its 4 sentences
# into 4 fixed-size slots (max length per slot across cores, regions rounded to
# 128) so that all 8 cores execute one identical SPMD program. Padded rows are
# zeros; softmax denominators are corrected by subtracting the per-core pad
# count (pad keys contribute exp(0)=1 exactly), shipped as data.
#
# Precision: matmul operands in bf16 (full PE rate; fp32 matmul is 4x slower),
# accumulation in fp32 PSUM, softmax sum / residual / layernorm in fp32.
import sys

for _p in ("/opt/trn_rl_repo", "/root/.axon_site/_ro/trn_rl_repo"):
    if _p not in sys.path:
        sys.path.insert(0, _p)

import numpy as np
import ml_dtypes

import concourse.bass as bass  # noqa: F401  (bass types used via bacc/tile)
import concourse.mybir as mybir
import concourse.tile as tile
from concourse import bacc

BF16 = ml_dtypes.bfloat16
F32 = np.float32

N_CORES = 8
MB = 32
D_MODEL = 1024
D_HALF = 512  # d_content == d_pos
N_HEAD = 8
D_K = 128
DK2 = 64
SCALE = float(D_MODEL) ** 0.5  # 32.0
EPS = 1e-3
P = 128  # partitions


def _ceil_to(x, m):
    return (x + m - 1) // m * m


class Plan:
    def __init__(self, lengths):
        lengths = np.asarray(lengths, np.int64)
        assert lengths.shape == (MB,)
        order = np.argsort(-lengths, kind="stable")
        # core c handles sentence ranks {c, 15-c, 16+c, 31-c} (desc length order)
        self.core_sents = [
            [int(order[c]), int(order[15 - c]), int(order[16 + c]), int(order[31 - c])]
            for c in range(N_CORES)
        ]
        self.lengths = lengths
        self.slot_pad = [
            max(int(lengths[self.core_sents[c][j]]) for c in range(N_CORES))
            for j in range(4)
        ]
        self.regions = [_ceil_to(sp, P) for sp in self.slot_pad]
        self.offs = [0]
        for r in self.regions[:-1]:
            self.offs.append(self.offs[-1] + r)
        self.t_pad = sum(self.regions)
        assert self.t_pad % P == 0
        self.nt = self.t_pad // P
        self.glob_off = np.concatenate([[0], np.cumsum(lengths)[:-1]]).astype(np.int64)

    @property
    def key(self):
        return (tuple(self.slot_pad), self.t_pad)



def _copy(nc, eng, out, in_):
    # engine-dispatched copy: DVE has tensor_copy, ACT uses activation(Copy)
    if eng is nc.scalar:
        nc.scalar.copy(out, in_)
    else:
        eng.tensor_copy(out, in_)

def _build_program(plan: Plan):
    """Build and compile the single-core Bass program (same for all cores)."""
    T = plan.t_pad
    NT = plan.nt
    nc = bacc.Bacc("TRN2", target_bir_lowering=False, debug=False)

    dt = mybir.dt
    # ---- DRAM I/O ----
    xT_d = nc.dram_tensor("xT", [D_MODEL, T], dt.bfloat16, kind="ExternalInput").ap()
    x_d = nc.dram_tensor("x", [T, D_MODEL], dt.float32, kind="ExternalInput").ap()
    wq_d = nc.dram_tensor("wq", [4, 8, P, P], dt.bfloat16, kind="ExternalInput").ap()
    wk_d = nc.dram_tensor("wk", [4, 8, P, P], dt.bfloat16, kind="ExternalInput").ap()
    wv_d = nc.dram_tensor("wv", [8, P, D_HALF], dt.bfloat16, kind="ExternalInput").ap()
    pw_d = nc.dram_tensor("pw", [2, 4, P, D_HALF], dt.bfloat16, kind="ExternalInput").ap()
    npad_d = nc.dram_tensor("npad", [P, 4], dt.float32, kind="ExternalInput").ap()
    ident_d = nc.dram_tensor("ident", [P, P], dt.bfloat16, kind="ExternalInput").ap()
    out_d = nc.dram_tensor("out", [T, D_MODEL], dt.float32, kind="ExternalOutput").ap()

    with tile.TileContext(nc) as tc:
        with (
            tc.tile_pool(name="persist", bufs=1) as pp,
            tc.tile_pool(name="weights", bufs=1) as wp,
        ):
            # persistent SBUF tensors
            qt = pp.tile([P, N_HEAD, T], dt.bfloat16, tag="qt")   # Q^T per head
            kt = pp.tile([P, N_HEAD, T], dt.bfloat16, tag="kt")   # K^T per head
            # V token-natural, head-major columns: [p, tile, head, {c64|p64}]
            vv = pp.tile([P, NT, D_MODEL], dt.bfloat16, tag="vv")
            o1t = pp.tile([P, 4, T], dt.bfloat16, tag="o1t")      # o1^T chunk-major
            o2t = pp.tile([P, 4, T], dt.bfloat16, tag="o2t")
            vv_w = vv.rearrange("p t (h b d) -> p t h b d", h=N_HEAD, b=2)
            npad_sb = pp.tile([P, 4], dt.float32, tag="npad")
            ident_sb = pp.tile([P, P], dt.bfloat16, tag="ident")

            wq_sb = wp.tile([P, 4, 8, P], dt.bfloat16, tag="wq")
            wk_sb = wp.tile([P, 4, 8, P], dt.bfloat16, tag="wk")
            wv_sb = wp.tile([P, 8, D_HALF], dt.bfloat16, tag="wv")
            pw_sb = wp.tile([P, 2, 4, D_HALF], dt.bfloat16, tag="pw")

            nc.sync.dma_start(npad_sb[:, :], npad_d[:, :])
            nc.sync.dma_start(ident_sb[:, :], ident_d[:, :])
            for pr in range(4):
                for j in range(8):
                    nc.sync.dma_start(wq_sb[:, pr, j, :], wq_d[pr, j, :, :])
                    nc.sync.dma_start(wk_sb[:, pr, j, :], wk_d[pr, j, :, :])
            for j in range(8):
                nc.sync.dma_start(wv_sb[:, j, :], wv_d[j, :, :])
            for i in range(2):
                for k in range(4):
                    nc.sync.dma_start(pw_sb[:, i, k, :], pw_d[i, k, :, :])

            # zero the attention-output staging (pad-query columns are never
            # written; keep them finite for the projection matmuls)
            nc.vector.memset(o1t[:, :, :], 0.0)
            nc.vector.memset(o2t[:, :, :], 0.0)

            # ================= Phase 1: QKV projections =================
            GW = 512  # token group width for Q/K matmuls
            with (
                tc.tile_pool(name="xt_pool", bufs=2) as xtp,
                tc.tile_pool(name="qk_ps", bufs=4, space="PSUM") as qkps,
                tc.tile_pool(name="v_ps", bufs=3, space="PSUM") as vps,
            ):
                for g0 in range(0, T, GW):
                    gw = min(GW, T - g0)
                    xt_sb = xtp.tile([P, 8, GW], dt.bfloat16, tag="xt")
                    for ch in range(8):
                        nc.sync.dma_start(
                            xt_sb[:, ch, 0:gw], xT_d[ch * P:(ch + 1) * P, g0:g0 + gw]
                        )
                    # --- Q/K head-pair matmuls over the whole group ---
                    for pr in range(4):
                        h0, h1 = 2 * pr, 2 * pr + 1
                        for comp, (w_sb, dst) in enumerate(
                            ((wq_sb, qt), (wk_sb, kt))
                        ):
                            for half in range(2):  # 0: content dims, 1: pos dims
                                acc = qkps.tile([P, GW], dt.float32, tag="qkacc")
                                for jj in range(4):
                                    j = half * 4 + jj
                                    nc.tensor.matmul(
                                        acc[:, 0:gw],
                                        w_sb[:, pr, j, :],
                                        xt_sb[:, j, 0:gw],
                                        start=(jj == 0),
                                        stop=(jj == 3),
                                    )
                                # distribute: rows 0:64 -> head h0, 64:128 -> h1
                                eng = nc.vector if comp == 0 else nc.scalar
                                eng2 = nc.scalar if comp == 0 else nc.vector
                                _copy(nc, eng,
                                      dst[64 * half:64 * half + 64, h0, g0:g0 + gw],
                                      acc[0:64, 0:gw])
                                _copy(nc, eng2,
                                      dst[64 * half:64 * half + 64, h1, g0:g0 + gw],
                                      acc[64:128, 0:gw])
                    # --- V matmuls (token tiles of 128) ---
                    for tt in range(g0 // P, (g0 + gw) // P):
                        tl = tt * P - g0
                        for half in range(2):
                            vacc = vps.tile([P, D_HALF], dt.float32, tag="vacc")
                            vacc_r = vacc.rearrange("p (h d) -> p h d", h=N_HEAD)
                            for jj in range(4):
                                j = half * 4 + jj
                                nc.tensor.matmul(
                                    vacc[:, :],
                                    xt_sb[:, j, tl:tl + P],
                                    wv_sb[:, j, :],
                                    start=(jj == 0),
                                    stop=(jj == 3),
                                )
                            # scatter head h's 64 cols to h*128 + half*64
                            nc.scalar.copy(
                                vv_w[:, tt, :, half, :],
                                vacc_r[:, :, :],
                            )

            vv_r = vv.rearrange("p t (h d) -> p t h d", h=N_HEAD)

            # ================= Phase 2: attention =================
            with (
                tc.tile_pool(name="lg_ps", bufs=3, space="PSUM") as lgps,
                tc.tile_pool(name="at_ps", bufs=3, space="PSUM") as atps,
                tc.tile_pool(name="ot_ps", bufs=2, space="PSUM") as otps,
                tc.tile_pool(name="attn_sb", bufs=3) as asb,
                tc.tile_pool(name="small_sb", bufs=4) as ssb,
            ):
                for s in range(4):
                    L = plan.slot_pad[s]
                    koff = plan.offs[s]
                    nk = (L + P - 1) // P
                    for h in range(N_HEAD):
                        half, pr = h % 2, h // 2
                        for qi in range(nk):
                            qoff = koff + P * qi
                            lq = min(P, L - P * qi)
                            lg = lgps.tile([P, 512], dt.float32, tag="lg")
                            nc.tensor.matmul(
                                lg[0:lq, 0:L],
                                qt[:, h, qoff:qoff + lq],
                                kt[:, h, koff:koff + L],
                                start=True,
                                stop=True,
                            )
                            attn = asb.tile([P, 512], dt.bfloat16, tag="attn")
                            se = ssb.tile([P, 1], dt.float32, tag="se")
                            nc.scalar.activation(
                                attn[0:lq, 0:L],
                                lg[0:lq, 0:L],
                                mybir.ActivationFunctionType.Exp,
                                scale=1.0 / SCALE,
                                accum_out=se[0:lq, :],
                            )
                            rc = ssb.tile([P, 1], dt.float32, tag="rc")
                            nc.vector.tensor_tensor(
                                rc[0:lq, :], se[0:lq, :], npad_sb[0:lq, s:s + 1],
                                mybir.AluOpType.subtract,
                            )
                            nc.vector.reciprocal(rc[0:lq, :], rc[0:lq, :])
                            diag = ssb.tile([P, P], dt.bfloat16, tag="diag")
                            nc.vector.tensor_scalar(
                                diag[0:lq, 0:lq], ident_sb[0:lq, 0:lq],
                                rc[0:lq, :], None, mybir.AluOpType.mult,
                            )
                            ot = otps.tile([P, P], dt.float32, tag="ot")
                            for ki in range(nk):
                                kc = min(P, L - P * ki)
                                # attn^T * diag(recip): transpose + normalize
                                at_ps = atps.tile([P, P], dt.float32, tag="atp")
                                nc.tensor.matmul(
                                    at_ps[0:kc, 0:lq],
                                    attn[0:lq, P * ki:P * ki + kc],
                                    diag[0:lq, 0:lq],
                                    start=True,
                                    stop=True,
                                )
                                at_sb = asb.tile([P, P], dt.bfloat16, tag="at_sb")
                                eng = nc.vector if (ki % 2 == 0) else nc.scalar
                                _copy(nc, eng,
                                      at_sb[0:kc, 0:lq], at_ps[0:kc, 0:lq])
                                nc.tensor.matmul(
                                    ot[:, 0:lq],
                                    vv_r[0:kc, koff // P + ki, h, :],
                                    at_sb[0:kc, 0:lq],
                                    start=(ki == 0),
                                    stop=(ki == nk - 1),
                                )
                            # distribute attention output (rows 0:64 content,
                            # 64:128 pos) into proj-ready chunk-major layout
                            nc.vector.tensor_copy(
                                o1t[64 * half:64 * half + 64, pr, qoff:qoff + lq],
                                ot[0:64, 0:lq],
                            )
                            nc.vector.tensor_copy(
                                o2t[64 * half:64 * half + 64, pr, qoff:qoff + lq],
                                ot[64:128, 0:lq],
                            )

            # ================= Phase 3: projections + residual + layernorm ====
            with (
                tc.tile_pool(name="z_ps", bufs=4, space="PSUM") as zps,
                tc.tile_pool(name="z_sb", bufs=3) as zsb,
                tc.tile_pool(name="x_sb", bufs=3) as xsb,
                tc.tile_pool(name="ln_sb", bufs=4) as lsb,
            ):
                for t in range(NT):
                    t0 = t * P
                    zh = []
                    for i, osrc in enumerate((o1t, o2t)):
                        zp = zps.tile([P, D_HALF], dt.float32, tag="zp")
                        for k in range(4):
                            nc.tensor.matmul(
                                zp[:, :],
                                osrc[:, k, t0:t0 + P],
                                pw_sb[:, i, k, :],
                                start=(k == 0),
                                stop=(k == 3),
                            )
                        zh.append(zp)
                    xt_f = xsb.tile([P, D_MODEL], dt.float32, tag="xf")
                    nc.sync.dma_start(xt_f[:, :], x_d[t0:t0 + P, :])
                    z = zsb.tile([P, D_MODEL], dt.float32, tag="z")
                    for i in range(2):
                        nc.vector.tensor_tensor(
                            z[:, i * D_HALF:(i + 1) * D_HALF],
                            zh[i][:, :],
                            xt_f[:, i * D_HALF:(i + 1) * D_HALF],
                            mybir.AluOpType.add,
                        )
                    # layernorm stats: mu, unbiased sigma (eps added to sigma)
                    zsum = lsb.tile([P, 1], dt.float32, tag="zsum")
                    nc.vector.reduce_sum(
                        zsum[:, :], z[:, :], axis=mybir.AxisListType.X
                    )
                    sq = zsb.tile([P, D_MODEL], dt.float32, tag="sq")
                    ssq = lsb.tile([P, 1], dt.float32, tag="ssq")
                    nc.scalar.activation(
                        sq[:, :], z[:, :], mybir.ActivationFunctionType.Square,
                        accum_out=ssq[:, :],
                    )
                    mu = lsb.tile([P, 1], dt.float32, tag="mu")
                    nc.vector.tensor_scalar(
                        mu[:, :], zsum[:, :], 1.0 / D_MODEL, None,
                        mybir.AluOpType.mult,
                    )
                    var = lsb.tile([P, 1], dt.float32, tag="var")
                    nc.vector.tensor_tensor(
                        var[:, :], zsum[:, :], mu[:, :], mybir.AluOpType.mult
                    )
                    nc.vector.tensor_tensor(
                        var[:, :], ssq[:, :], var[:, :], mybir.AluOpType.subtract
                    )
                    sig = lsb.tile([P, 1], dt.float32, tag="sig")
                    nc.scalar.activation(
                        sig[:, :], var[:, :], mybir.ActivationFunctionType.Sqrt,
                        scale=1.0 / (D_MODEL - 1),
                    )
                    nc.vector.tensor_scalar(
                        sig[:, :], sig[:, :], EPS, None, mybir.AluOpType.add
                    )
                    rstd = lsb.tile([P, 1], dt.float32, tag="rstd")
                    nc.vector.reciprocal(rstd[:, :], sig[:, :])
                    negmu = lsb.tile([P, 1], dt.float32, tag="negmu")
                    nc.vector.tensor_scalar(
                        negmu[:, :], zsum[:, :], -1.0 / D_MODEL, None,
                        mybir.AluOpType.mult,
                    )
                    o = zsb.tile([P, D_MODEL], dt.float32, tag="o")
                    nc.vector.tensor_scalar(
                        o[:, :], z[:, :], negmu[:, :], rstd[:, :],
                        mybir.AluOpType.add, mybir.AluOpType.mult,
                    )
                    nc.sync.dma_start(out_d[t0:t0 + P, :], o[:, :])

    nc.compile()
    return nc


_PROGRAMS = {}   # plan.key -> (nc, plan)
_RUNNERS = {}    # plan.key -> callable(in_maps) -> list[dict]


def _get_program(plan: Plan):
    if plan.key not in _PROGRAMS:
        _PROGRAMS[plan.key] = _build_program(plan)
    return _PROGRAMS[plan.key]


def _make_runner(nc, donate=True):
    """Cached PJRT runner (mirrors bass_utils.run_bass_kernel_spmd's axon
    path via bass2jax, but reuses the jitted executable across calls)."""
    import jax
    from jax.sharding import Mesh, PartitionSpec
    from jax.experimental.shard_map import shard_map
    from concourse import bass2jax

    bass2jax.install_neuronx_cc_hook()

    partition_name = (nc.partition_id_tensor.name
                      if nc.partition_id_tensor else None)
    in_names, out_names, out_avals, zero_shapes = [], [], [], []
    for alloc in nc.m.functions[0].allocations:
        if not isinstance(alloc, mybir.MemoryLocationSet):
            continue
        name = alloc.memorylocations[0].name
        if alloc.kind == "ExternalInput":
            if name == partition_name:
                continue
            in_names.append(name)
        elif alloc.kind == "ExternalOutput":
            out_names.append(name)
            shape = tuple(alloc.tensor_shape)
            dtype = mybir.dt.np(alloc.dtype)
            out_avals.append(jax.core.ShapedArray(shape, dtype))
            zero_shapes.append((shape, dtype))
    n_params = len(in_names)
    all_names = in_names + out_names
    if partition_name is not None:
        all_names = all_names + [partition_name]

    def _body(*args):
        operands = list(args)
        if partition_name is not None:
            operands.append(bass2jax.partition_id_tensor())
        outs = bass2jax._bass_exec_p.bind(
            *operands,
            out_avals=tuple(out_avals),
            in_names=tuple(all_names),
            out_names=tuple(out_names),
            lowering_input_output_aliases=(),
            sim_require_finite=True,
            sim_require_nnan=True,
            nc=nc,
        )
        return tuple(outs)

    devices = jax.devices()[:N_CORES]
    mesh = Mesh(np.asarray(devices), ("core",))
    in_specs = (PartitionSpec("core"),) * (n_params + len(out_names))
    out_specs = (PartitionSpec("core"),) * len(out_names)
    sharded = jax.jit(
        shard_map(_body, mesh=mesh, in_specs=in_specs, out_specs=out_specs,
                  check_rep=False),
        donate_argnums=tuple(range(n_params, n_params + len(out_names)))
        if donate else (),
        keep_unused=True,
    )

    def run(in_maps):
        concat_in = [
            np.concatenate([np.asarray(m[name]) for m in in_maps], axis=0)
            for name in in_names
        ]
        concat_zeros = [
            np.zeros((N_CORES * s[0], *s[1:]), d) for (s, d) in zero_shapes
        ]
        out_arrs = sharded(*concat_in, *concat_zeros)
        return [
            {
                name: np.asarray(out_arrs[i]).reshape(
                    N_CORES, *out_avals[i].shape)[c]
                for i, name in enumerate(out_names)
            }
            for c in range(N_CORES)
        ]

    run.sharded = sharded
    run.in_names = in_names
    run.out_names = out_names
    run.out_avals = out_avals
    run.zero_shapes = zero_shapes
    run.n_params = n_params
    return run


def _prep_weights(w_qs1, w_ks1, w_vs1, w_qs2, w_ks2, w_vs2, proj1_w, proj2_w):
    wq = np.zeros((4, 8, P, P), BF16)
    wk = np.zeros((4, 8, P, P), BF16)
    for pr in range(4):
        h0, h1 = 2 * pr, 2 * pr + 1
        for j in range(8):
            if j < 4:
                rows = slice(j * P, (j + 1) * P)
                wq[pr, j] = np.concatenate(
                    [w_qs1[h0, rows, :], w_qs1[h1, rows, :]], axis=1).astype(BF16)
                wk[pr, j] = np.concatenate(
                    [w_ks1[h0, rows, :], w_ks1[h1, rows, :]], axis=1).astype(BF16)
            else:
                rows = slice((j - 4) * P, (j - 3) * P)
                wq[pr, j] = np.concatenate(
                    [w_qs2[h0, rows, :], w_qs2[h1, rows, :]], axis=1).astype(BF16)
                wk[pr, j] = np.concatenate(
                    [w_ks2[h0, rows, :], w_ks2[h1, rows, :]], axis=1).astype(BF16)
    wv = np.zeros((8, P, D_HALF), BF16)
    for j in range(8):
        src = w_vs1 if j < 4 else w_vs2
        rows = slice((j % 4) * P, (j % 4 + 1) * P)
        wv[j] = np.concatenate([src[h, rows, :] for h in range(8)], axis=1
                               ).astype(BF16)
    pw = np.zeros((2, 4, P, D_HALF), BF16)
    p1T = np.ascontiguousarray(proj1_w.T)  # [in, out]
    p2T = np.ascontiguousarray(proj2_w.T)
    for k in range(4):
        pw[0, k] = p1T[k * P:(k + 1) * P, :].astype(BF16)
        pw[1, k] = p2T[k * P:(k + 1) * P, :].astype(BF16)
    return wq, wk, wv, pw


def _prep_core_inputs(plan: Plan, inp, c):
    T = plan.t_pad
    x = np.zeros((T, D_MODEL), F32)
    npad = np.zeros((4,), F32)
    for j in range(4):
        s = plan.core_sents[c][j]
        L = int(plan.lengths[s])
        g0 = int(plan.glob_off[s])
        x[plan.offs[j]:plan.offs[j] + L] = inp[g0:g0 + L]
        npad[j] = plan.slot_pad[j] - L
    xT = np.ascontiguousarray(x.T).astype(BF16)
    npad_rep = np.tile(npad[None, :], (P, 1)).astype(F32)
    return x, xT, npad_rep


def make_in_maps(plan: Plan, inp, weights):
    wq, wk, wv, pw = weights
    ident = np.eye(P, dtype=BF16)
    in_maps = []
    for c in range(N_CORES):
        x, xT, npad_rep = _prep_core_inputs(plan, inp, c)
        in_maps.append({
            "xT": xT, "x": x, "wq": wq, "wk": wk, "wv": wv, "pw": pw,
            "npad": npad_rep, "ident": ident,
        })
    return in_maps


def gather_output(plan: Plan, results, a_2=None, b_2=None):
    T_tot = int(plan.lengths.sum())
    out = np.empty((T_tot, D_MODEL), F32)
    for c in range(N_CORES):
        oc = results[c]["out"]
        for j in range(4):
            s = plan.core_sents[c][j]
            L = int(plan.lengths[s])
            g0 = int(plan.glob_off[s])
            out[g0:g0 + L] = oc[plan.offs[j]:plan.offs[j] + L]
    if a_2 is not None and (np.any(a_2 != 1.0) or np.any(b_2 != 0.0)):
        out = out * np.asarray(a_2, F32) + np.asarray(b_2, F32)
    return out


def kernel(inp, w_qs1, w_ks1, w_vs1, w_qs2, w_ks2, w_vs2,
           proj1_w, proj2_w, a_2, b_2, token_batch, token_pos, valid_mask):
    inp = np.asarray(inp, F32)
    token_batch = np.asarray(token_batch)
    lengths = np.bincount(token_batch, minlength=MB).astype(np.int64)
    # tokens of each sentence must be contiguous and in order
    plan = Plan(lengths)

    nc = _get_program(plan)
    if plan.key not in _RUNNERS:
        _RUNNERS[plan.key] = _make_runner(nc)
    runner = _RUNNERS[plan.key]

    weights = _prep_weights(np.asarray(w_qs1), np.asarray(w_ks1),
                            np.asarray(w_vs1), np.asarray(w_qs2),
                            np.asarray(w_ks2), np.asarray(w_vs2),
                            np.asarray(proj1_w), np.asarray(proj2_w))
    in_maps = make_in_maps(plan, inp, weights)
    results = runner(in_maps)
    return gather_output(plan, results, np.asarray(a_2), np.asarray(b_2))
